# revision 37
# baseline (speedup 1.0000x reference)
"""Trainium2 Bass kernel for a dense transformer block (pre-LN GPT block).

Reference computation (fp32, B=2, T=2048, C=1024, H=16 heads, FFN 4C):
    x = x + attn(LN1(x)) ; x = x + mlp(LN2(x))   (causal attention, tanh-gelu)

Distribution (8 NeuronCores, no collectives):
  - batch split (2) x sequence split (4): core c handles batch b=c//4,
    query quarter j=c%4 (512 tokens).
  - K/V projections are computed for the full 2048-token batch on every
    core of the group (replicated); everything else (Q, attention rows,
    proj, LN2, FFN, residuals) is token-local.
  - causality via host-built masks: tokens are rotated per-core so the
    own 512 tokens come first in the key order; the 512x512 diagonal
    block uses on-the-fly iota masks; the remaining key tiles are
    uniformly allowed/denied per core by zeroing V rows (and their
    softmax-denominator ones-column) via the m01 input.

Precision strategy: all projection matmuls (QKV, attn-proj, FFN fc and
proj) run in fp8e4 with DoubleRow perf mode (two 128-row contraction
tiles per instruction); the AV matmul likewise (A=exp(s-3) and V are
quantized to fp8e4; the -3 bias keeps A under fp8e4's 240 max and
cancels in the softmax ratio because the denominator rides as a 65th
ones-column of V through the same fp8 pipeline). Score matmuls (QK^T)
stay bf16 for softmax accuracy. Weights are pre-scaled by powers of two
on the host so fp8 sees unit-variance values; the inverse scales fold
into existing per-tile scalar passes. LN is folded into the following
matmul via augmented contraction rows; contraction is padded 9->10
tiles (5 DoubleRow pairs).
"""

import math
import numpy as np
import ml_dtypes

B, T, C = 2, 2048, 1024
H, DH = 16, 64
F = 4 * C
Q = 512          # query tokens per core
NCORES = 8
KT = T // 128    # 16 key tiles
CT = C // 128    # 8 feature tiles
CP = CT // 2     # 4 feature pair-tiles
AUGP = 5         # contraction pair-tiles incl. LN-fold aug pair (10 x 128)
FT = F // 128    # 32 ffn tiles
FP = FT // 2     # 16 ffn pair-tiles
LN_EPS = 1e-5
NEG = -30000.0
EXP_BIAS = -3.0  # exp(s-3): keeps A < fp8e4 max; cancels in softmax ratio
SW = 32.0        # host pre-scale on wk/wv/wp/wf (power of 2)
SWQ = 256.0      # host pre-scale on wq (carries 1/sqrt(DH))
SWO = 64.0       # host pre-scale on wo

_cache = {}


def _build():
    import concourse.mybir as mybir
    import concourse.tile as tile
    from concourse import bacc

    f32 = mybir.dt.float32
    bf16 = mybir.dt.bfloat16
    f8 = mybir.dt.float8e4
    Alu = mybir.AluOpType
    Act = mybir.ActivationFunctionType
    DR = mybir.MatmulPerfMode.DoubleRow

    nc = bacc.Bacc("TRN2", target_bir_lowering=False, debug=False,
                   num_devices=NCORES)

    xT_d = nc.dram_tensor("xT", [C, Q], bf16, kind="ExternalInput")
    xh_d = nc.dram_tensor("xh", [AUGP * 256, T], f8, kind="ExternalInput")
    xho_d = nc.dram_tensor("xho", [AUGP * 256, Q], f8, kind="ExternalInput")
    wq_d = nc.dram_tensor("wq", [AUGP * 256, C], f8, kind="ExternalInput")
    wk_d = nc.dram_tensor("wk", [AUGP * 256, C], f8, kind="ExternalInput")
    wv_d = nc.dram_tensor("wv", [AUGP * 256, C], f8, kind="ExternalInput")
    wp_d = nc.dram_tensor("wp", [C, C], f8, kind="ExternalInput")
    wf_d = nc.dram_tensor("wf", [(2 * CP + 1) * 128, F], bf16,
                          kind="ExternalInput")
    woh_d = nc.dram_tensor("woh", [F, C], f8, kind="ExternalInput")
    wol_d = nc.dram_tensor("wol", [F, C], f8, kind="ExternalInput")
    mskT_d = nc.dram_tensor("mskT", [128, KT * 128], mybir.dt.float8e5,
                            kind="ExternalInput")
    id_d = nc.dram_tensor("id128", [128, 128], f8, kind="ExternalInput")
    out_d = nc.dram_tensor("outT", [C, Q], f32, kind="ExternalOutput")

    # suffix trim: key tile kt feeds query columns [QS[kt], Q)
    QS = [128 * (kt // 4) for kt in range(KT)]

    with tile.TileContext(nc) as tc:
        cst = tc.alloc_tile_pool(name="cst", bufs=1, side="left")
        ones_col = cst.tile([128, 1], bf16, name="ones_col", tag="ones_col")
        ones_r128 = cst.tile([1, 128], f32, name="ones_r128", tag="ones_r128")
        ones_r64b = cst.tile([1, 64], bf16, name="ones_r64b", tag="ones_r64b")
        eps_t = cst.tile([1, 1], f32, name="eps", tag="eps")
        expb_t = cst.tile([128, 1], f32, name="expb", tag="expb")
        nc.vector.memset(ones_col[:], 1.0)
        nc.vector.memset(ones_r128[:], 1.0)
        nc.vector.memset(ones_r64b[:], 1.0)
        nc.vector.memset(eps_t[:], LN_EPS)
        nc.vector.memset(expb_t[:], EXP_BIAS)

        p_ytil = tc.alloc_tile_pool(name="ytil", bufs=1, side="left")
        ytil = [p_ytil.tile([128, 2, Q], f8, name=f"ytil{p}", tag=f"ytil{p}")
                for p in range(CP)]

        kqv = tc.alloc_tile_pool(name="kqv", bufs=1, side="left")
        kT_sb = [kqv.tile([128, T], bf16, name=f"kT{m}", tag=f"kT{m}")
                 for m in range(CT)]
        qT_sb = [kqv.tile([128, Q], bf16, name=f"qT{m}", tag=f"qT{m}")
                 for m in range(CT)]
        v_sb = [kqv.tile([128, 2, H, DH + 1], f8, name=f"v{t}", tag=f"v{t}")
                for t in range(KT // 2)]
        mskT_sb = kqv.tile([128, KT * 128], mybir.dt.float8e5, name="mskT",
                           tag="mskT")
        id_sb = kqv.tile([128, 128], f8, name="id128", tag="id128")
        # softmax-denominator ones column (masked keys zero out via exp->0)
        for tp in range(KT // 2):
            nc.vector.memset(v_sb[tp][:, :, :, DH:DH + 1], 1.0)

        p_xhat = tc.alloc_tile_pool(name="xhat", bufs=1, side="left")
        xhat = [p_xhat.tile([128, 2, T], f8, name=f"xh{k}", tag=f"xh{k}")
                for k in range(AUGP)]
        xho_sb = [p_xhat.tile([128, 2, Q], f8, name=f"xho{k}", tag=f"xho{k}")
                  for k in range(AUGP)]

        # QKV weights (left, release order: wv -> wq -> wk)
        p_wk = tc.alloc_tile_pool(name="wkp", bufs=1, side="left")
        wk_sb = [p_wk.tile([128, 2, C], f8, name=f"wk{k}", tag=f"wk{k}")
                 for k in range(AUGP)]
        p_wq = tc.alloc_tile_pool(name="wqp", bufs=1, side="left")
        wq_sb = [p_wq.tile([128, 2, C], f8, name=f"wq{k}", tag=f"wq{k}")
                 for k in range(AUGP)]
        p_wv = tc.alloc_tile_pool(name="wvp", bufs=1, side="left")
        wv_sb = [p_wv.tile([128, 2, C], f8, name=f"wv{k}", tag=f"wv{k}")
                 for k in range(AUGP)]

        # proj weights (right): loaded up front, consumed in phase 3
        p_wp = tc.alloc_tile_pool(name="wpp", bufs=1, side="right")
        wp_sb = [p_wp.tile([128, 2, C], f8, name=f"wp{k}", tag=f"wp{k}")
                 for k in range(CP)]

        # input DMA in priority order for the first attention pair:
        # xh -> wk/wq m=0 column slices -> xho -> masks -> wv head-half 0,
        # then the remainders
        for k in range(AUGP):
            for i in range(2):
                r0 = k * 256 + i * 128
                nc.sync.dma_start(xhat[k][:, i, :], xh_d[r0:r0 + 128, :])
        for k in range(AUGP):
            for i in range(2):
                r0 = k * 256 + i * 128
                nc.sync.dma_start(wk_sb[k][:, i, 0:128], wk_d[r0:r0 + 128, 0:128])
                nc.sync.dma_start(wq_sb[k][:, i, 0:128], wq_d[r0:r0 + 128, 0:128])
                nc.sync.dma_start(xho_sb[k][:, i, :], xho_d[r0:r0 + 128, :])
        nc.sync.dma_start(mskT_sb[:], mskT_d[:])
        nc.sync.dma_start(id_sb[:], id_d[:])
        for k in range(AUGP):
            for i in range(2):
                r0 = k * 256 + i * 128
                nc.sync.dma_start(wv_sb[k][:, i, 0:512], wv_d[r0:r0 + 128, 0:512])
        for k in range(AUGP):
            for i in range(2):
                r0 = k * 256 + i * 128
                nc.sync.dma_start(wk_sb[k][:, i, 128:], wk_d[r0:r0 + 128, 128:])
                nc.sync.dma_start(wq_sb[k][:, i, 128:], wq_d[r0:r0 + 128, 128:])
        for k in range(AUGP):
            for i in range(2):
                r0 = k * 256 + i * 128
                nc.sync.dma_start(wv_sb[k][:, i, 512:], wv_d[r0:r0 + 128, 512:])
        for k in range(CP):
            for i in range(2):
                r0 = k * 256 + i * 128
                nc.sync.dma_start(wp_sb[k][:, i, :], wp_d[r0:r0 + 128, :])

        # ffn weights, part 1: allocated up front so the DMA can run during
        # the attention tail (6 of 9 bf16 tiles; the rest load in phase 3)
        AUG9 = 2 * CP + 1
        p_wf1 = tc.alloc_tile_pool(name="wfp1", bufs=1, side="right")
        wf1 = [p_wf1.tile([128, F], bf16, name=f"wf{k}", tag=f"wf{k}")
               for k in range(6)]

        # ---- attention: K/Q/V projections pipelined into the head loop ----
        with tc.tile_pool(name="pa", bufs=4, side="right") as p_a, \
             tc.tile_pool(name="prl", bufs=1, side="right") as p_rl, \
             tc.tile_pool(name="pqkv", bufs=2, space="PSUM") as pq, \
             tc.tile_pool(name="ps2", bufs=2, space="PSUM") as ps2, \
             tc.tile_pool(name="py", bufs=2, space="PSUM") as py:

            def v_chunk(n, t):
                # V proj of key tile t for head-half n (v dims [n*512,+512))
                ns = slice(n * 512, (n + 1) * 512)
                ts_ = slice(t * 128, (t + 1) * 128)
                ps = pq.tile([128, 8, 64], f32, name="pk", tag="pk")
                for k in range(AUGP):
                    nc.tensor.matmul(ps[:], xhat[k][:, :, ts_],
                                     wv_sb[k][:, :, ns],
                                     start=(k == 0), stop=(k == AUGP - 1),
                                     perf_mode=DR)
                nc.vector.tensor_scalar_mul(
                    v_sb[t // 2][:, t % 2, n * 8:(n + 1) * 8, 0:DH], ps[:],
                    1.0 / SW)

            def k_chunk(m, n):
                ns = slice(n * 512, (n + 1) * 512)
                ps = pq.tile([128, 512], f32, name="pk", tag="pk")
                for k in range(AUGP):
                    nc.tensor.matmul(ps[:], wk_sb[k][:, :, m * 128:(m + 1) * 128],
                                     xhat[k][:, :, ns],
                                     start=(k == 0), stop=(k == AUGP - 1),
                                     perf_mode=DR)
                nc.vector.tensor_scalar_mul(kT_sb[m][:, ns], ps[:], 1.0 / SW)

            def q_chunk(m):
                ps = pq.tile([128, 512], f32, name="pk", tag="pk")
                for k in range(AUGP):
                    nc.tensor.matmul(ps[:], wq_sb[k][:, :, m * 128:(m + 1) * 128],
                                     xho_sb[k][:],
                                     start=(k == 0), stop=(k == AUGP - 1),
                                     perf_mode=DR)
                nc.vector.tensor_scalar_mul(qT_sb[m][:], ps[:], 1.0 / SWQ)

            pending = []

            def pump():
                if pending:
                    pending.pop(0)()

            def attention_head(h):
                kt_tile = h // 2
                po = (h % 2) * 64
                yb = py.tile([128, 512], f32, name="y", tag="y")
                for tp in range(KT // 2):        # key-tile pairs
                    pump()
                    pump()
                    qs = QS[2 * tp]
                    s_ps = ps2.tile([128, 2, 512], f32, name="s", tag="s")
                    a_sb = p_a.tile([128, 2, 512], f8, name="a", tag="a")
                    for half in range(2):
                        t = tp * 2 + half
                        # leading 128-col block: host-built causal/padding
                        # mask lands in psum via a tiny matmul, then scores
                        # accumulate on top; the suffix is mask-free.
                        nc.tensor.matmul(
                            s_ps[:, half, qs:qs + 128],
                            mskT_sb[:, t * 128:(t + 1) * 128], id_sb[:],
                            start=True, stop=False, skip_group_check=True)
                        nc.tensor.matmul(
                            s_ps[:, half, qs:qs + 128],
                            kT_sb[kt_tile][po:po + 64, t * 128:(t + 1) * 128],
                            qT_sb[kt_tile][po:po + 64, qs:qs + 128],
                            start=False, stop=True, skip_group_check=True)
                        if qs + 128 < Q:
                            nc.tensor.matmul(
                                s_ps[:, half, qs + 128:],
                                kT_sb[kt_tile][po:po + 64,
                                               t * 128:(t + 1) * 128],
                                qT_sb[kt_tile][po:po + 64, qs + 128:],
                                start=True, stop=True, skip_group_check=True)
                    nc.scalar.activation(a_sb[:, :, qs:], s_ps[:, :, qs:],
                                         Act.Exp, bias=expb_t[:])
                    nc.tensor.matmul(yb[0:65, qs:], v_sb[tp][:, :, h, :],
                                     a_sb[:, :, qs:],
                                     start=(tp == 0), stop=(tp == KT // 2 - 1),
                                     perf_mode=DR, skip_group_check=True)
                rl = p_rl.tile([1, 512], bf16, name="rl", tag="rl")
                rlf = p_rl.tile([1, 512], f32, name="rlf", tag="rlf")
                nc.vector.reciprocal(rlf[:], yb[64:65, :])
                nc.vector.tensor_copy(rl[:], rlf[:])
                nc.tensor.matmul(yb[64:128, :], ones_r64b[:], rl[:],
                                 start=True, stop=True)
                rlb = p_rl.tile([64, 512], bf16, name="rlb", tag="rlb")
                nc.vector.tensor_copy(rlb[:], yb[64:128, :])
                # head h -> ytil pair p=h//4, slot (h//2)%2, rows 64*(h%2)
                nc.vector.tensor_tensor(
                    ytil[h // 4][64 * (h % 2):64 * (h % 2) + 64,
                                 (h // 2) % 2, :],
                    yb[0:64, :], rlb[:], Alu.mult)

            # prologue: K/Q for m=0 (gating head 0), then first V0 tiles
            for n in range(4):
                k_chunk(0, n)
            q_chunk(0)
            for t in range(4):
                v_chunk(0, t)
            pending += [lambda t=t: v_chunk(0, t) for t in range(4, KT)]
            V1_SCHED = {1: range(0, 6), 2: range(6, 12), 3: range(12, 16)}
            for m in range(CT):
                if m < CT - 1:
                    pending.extend(
                        [lambda n=n, m1=m + 1: k_chunk(m1, n) for n in range(4)]
                        + [lambda m1=m + 1: q_chunk(m1)])
                for t in V1_SCHED.get(m, ()):
                    pending.append(lambda t=t: v_chunk(1, t))
                attention_head(2 * m)
                attention_head(2 * m + 1)
                while pending:
                    pending.pop(0)()
                if m == 4:
                    # prefetch most of wf (bf16, 6 of 9 tiles) while the
                    # tail attention iterations run
                    for k in range(6):
                        nc.sync.dma_start(wf1[k][:],
                                          wf_d[k * 128:(k + 1) * 128, :])
        p_wv.release()
        p_wq.release()
        p_wk.release()
        p_xhat.release()
        kqv.release()

        # ffn weights, part 2 (tiles 6-8): loaded during phase 3
        p_wf2 = tc.alloc_tile_pool(name="wfp2", bufs=1, side="right")
        wf_sb = wf1 + [p_wf2.tile([128, F], bf16, name=f"wf{k}", tag=f"wf{k}")
                       for k in range(6, AUG9)]

        # ------------ phase 3: proj + residual + LN2 ------------
        with tc.tile_pool(name="p34", bufs=1, side="right") as p34, \
             tc.tile_pool(name="p3s", bufs=2, side="right") as p3s:
            x2_sb = [p34.tile([128, Q], f32, name=f"x2{m}", tag=f"x2{m}")
                     for m in range(CT)]
            x2b = [p34.tile([128, Q], bf16, name=f"x2b{m}", tag=f"x2b{m}")
                   for m in range(CT)]
            xh2a = p34.tile([128, Q], bf16, name="xh2a", tag="xh2a")
            mu2 = p34.tile([1, Q], f32, name="mu2", tag="mu2")
            e22 = p34.tile([1, Q], f32, name="e22", tag="e22")
            rr2 = p34.tile([1, Q], f32, name="rr2", tag="rr2")
            mur2 = p34.tile([1, Q], f32, name="mur2", tag="mur2")
            r2b = p34.tile([128, Q], f32, name="r2b", tag="r2b")

            with tc.tile_pool(name="pxq", bufs=1, side="right") as p_xq:
                xq_sb = [p_xq.tile([128, Q], bf16, name=f"xq{m}", tag=f"xq{m}")
                         for m in range(CT)]
                for m in range(CT):
                    nc.sync.dma_start(xq_sb[m][:],
                                      xT_d[m * 128:(m + 1) * 128, :])
                for k in range(6, AUG9):
                    nc.sync.dma_start(wf_sb[k][:],
                                      wf_d[k * 128:(k + 1) * 128, :])
                with tc.tile_pool(name="pp3", bufs=4, space="PSUM") as pp3, \
                     tc.tile_pool(name="pst2", bufs=1, space="PSUM") as pst2:
                    s2_ps = pst2.tile([1, Q], f32, name="s2", tag="s2")
                    q2_ps = pst2.tile([1, Q], f32, name="q2", tag="q2")
                    for m in range(CT):
                        ms = slice(m * 128, (m + 1) * 128)
                        ps = pp3.tile([128, Q], f32, name="pj", tag="pj")
                        for k in range(CP):
                            nc.tensor.matmul(ps[:], wp_sb[k][:, :, ms],
                                             ytil[k][:],
                                             start=(k == 0), stop=(k == CP - 1),
                                             perf_mode=DR)
                        nc.vector.scalar_tensor_tensor(
                            x2_sb[m][:], ps[:], 1.0 / SW, xq_sb[m][:],
                            Alu.mult, Alu.add)
                        nc.vector.tensor_copy(x2b[m][:], x2_sb[m][:])
                        sqt = p3s.tile([128, Q], bf16, name="sq", tag="sq")
                        nc.scalar.square(sqt[:], x2b[m][:])
                        nc.tensor.matmul(s2_ps[:], ones_col[:], x2b[m][:],
                                         start=(m == 0), stop=(m == CT - 1))
                        nc.tensor.matmul(q2_ps[:], ones_col[:], sqt[:],
                                         start=(m == 0), stop=(m == CT - 1))
                    nc.vector.tensor_scalar_mul(mu2[:], s2_ps[:], 1.0 / C)
                    nc.vector.tensor_scalar_mul(e22[:], q2_ps[:], 1.0 / C)
            nc.vector.tensor_tensor(rr2[:], mu2[:], mu2[:], Alu.mult)
            nc.vector.tensor_tensor(rr2[:], e22[:], rr2[:], Alu.subtract)
            nc.scalar.activation(rr2[:], rr2[:], Act.Sqrt, bias=eps_t[:])
            nc.vector.reciprocal(rr2[:], rr2[:])
            nc.vector.tensor_tensor(mur2[:], mu2[:], rr2[:], Alu.mult)
            with tc.tile_pool(name="pbc2", bufs=1, space="PSUM") as pbc2:
                b_ps = pbc2.tile([128, Q], f32, name="b2", tag="b2")
                nc.tensor.matmul(b_ps[:], ones_r128[:], rr2[:],
                                 start=True, stop=True)
                nc.scalar.copy(r2b[:], b_ps[:])
            for k in range(CT):
                nc.vector.tensor_tensor(x2b[k][:], x2b[k][:], r2b[:], Alu.mult)
            nc.vector.memset(xh2a[:], 0.0)
            nc.vector.memset(xh2a[0:2, :], 1.0)
            nc.vector.tensor_copy(xh2a[0:1, :], mur2[:])
            xhat2 = x2b + [xh2a]

            # ------------ phase 4: FFN ------------
            # fc in bf16; gelu output split hg = hgh + hgl (both fp8e4) so
            # the fc-proj runs as three fp8 DoubleRow chains:
            #   o = hgh@(woh+wol) + hgl@woh   (wol = residual of woh quant)
            with tc.tile_pool(name="p4", bufs=1, side="right") as p4, \
                 tc.tile_pool(name="p4b", bufs=4, side="right") as p4b:
                hgh_sb = [p4.tile([128, 2, Q], f8, name=f"hgh{p}",
                                  tag=f"hgh{p}") for p in range(FP)]
                hgl_sb = [p4.tile([128, 2, Q], f8, name=f"hgl{p}",
                                  tag=f"hgl{p}") for p in range(FP)]
                with tc.tile_pool(name="ph", bufs=6, space="PSUM") as ph:
                    for m in range(FT):
                        ms = slice(m * 128, (m + 1) * 128)
                        ps = ph.tile([128, Q], f32, name="h", tag="h")
                        for k in range(AUG9):
                            nc.tensor.matmul(ps[:], wf_sb[k][:, ms],
                                             xhat2[k][:],
                                             start=(k == 0),
                                             stop=(k == AUG9 - 1))
                        hgb = p4b.tile([128, Q], bf16, name="hgb", tag="hgb")
                        nc.scalar.activation(hgb[:], ps[:],
                                             Act.Gelu_apprx_tanh)
                        hi = hgh_sb[m // 2][:, m % 2, :]
                        nc.vector.tensor_copy(hi, hgb[:])
                        nc.vector.scalar_tensor_tensor(
                            hgl_sb[m // 2][:, m % 2, :], hi, -1.0, hgb[:],
                            Alu.mult, Alu.add)
                with tc.tile_pool(name="pwo", bufs=4, side="right") as p_wo, \
                     tc.tile_pool(name="pwol", bufs=1, side="right") as p_wol, \
                     tc.tile_pool(name="pout", bufs=2, side="right") as p_out, \
                     tc.tile_pool(name="po", bufs=1, space="PSUM") as po:
                    o_ps = [po.tile([128, Q], f32, name=f"o{m}", tag=f"o{m}")
                            for m in range(CT)]
                    wol_t = [p_wol.tile([128, 2, C], f8, name=f"wol{kp}",
                                        tag=f"wol{kp}") for kp in range(FP)]
                    for kp in range(FP):
                        wo_t = p_wo.tile([128, 2, C], f8, name="wo", tag="wo")
                        for i in range(2):
                            r0 = kp * 256 + i * 128
                            nc.sync.dma_start(wo_t[:, i, :],
                                              woh_d[r0:r0 + 128, :])
                            nc.sync.dma_start(wol_t[kp][:, i, :],
                                              wol_d[r0:r0 + 128, :])
                        for m in range(CT):
                            nc.tensor.matmul(o_ps[m][:],
                                             wo_t[:, :, m * 128:(m + 1) * 128],
                                             hgh_sb[kp][:],
                                             start=(kp == 0), stop=False,
                                             perf_mode=DR,
                                             skip_group_check=True)
                        for m in range(CT):
                            nc.tensor.matmul(o_ps[m][:],
                                             wo_t[:, :, m * 128:(m + 1) * 128],
                                             hgl_sb[kp][:],
                                             start=False, stop=False,
                                             perf_mode=DR,
                                             skip_group_check=True)
                    # final chain m-outer so each output column block drains
                    # (residual add + store) while later blocks still matmul
                    for m in range(CT):
                        for kp in range(FP):
                            nc.tensor.matmul(o_ps[m][:],
                                             wol_t[kp][:, :,
                                                       m * 128:(m + 1) * 128],
                                             hgh_sb[kp][:],
                                             start=False, stop=(kp == FP - 1),
                                             perf_mode=DR,
                                             skip_group_check=True)
                        ot = p_out.tile([128, Q], f32, name="ot", tag="ot")
                        nc.vector.scalar_tensor_tensor(
                            ot[:], o_ps[m][:], 1.0 / SWO, x2_sb[m][:],
                            Alu.mult, Alu.add)
                        nc.sync.dma_start(out_d[m * 128:(m + 1) * 128, :], ot[:])

        p_wf2.release()
        p_wf1.release()
        p_wp.release()
        p_ytil.release()
        cst.release()

    nc.compile()
    return nc


def _prep_inputs(x, w_attn, w_proj, w_fc, w_fc_proj, ln1_w, ln1_b, ln2_w, ln2_b):
    f8 = ml_dtypes.float8_e4m3
    bf = ml_dtypes.bfloat16
    scale = 1.0 / math.sqrt(DH)

    def aug(W, lw, lb, s, rows, dt):
        out = np.zeros((rows, W.shape[1]), dtype=np.float32)
        Ws = lw[:, None] * W * s
        out[:C] = Ws
        out[C] = -Ws.sum(axis=0)
        out[C + 1] = (lb * s) @ W
        return out.astype(dt)

    wq = aug(w_attn[:, :C] * scale, ln1_w, ln1_b, SWQ, AUGP * 256, f8)
    wk = aug(w_attn[:, C:2 * C], ln1_w, ln1_b, SW, AUGP * 256, f8)
    wv = aug(w_attn[:, 2 * C:], ln1_w, ln1_b, SW, AUGP * 256, f8)
    wf = aug(w_fc, ln2_w, ln2_b, 1.0, (2 * CP + 1) * 128, bf)
    wp = (w_proj * SW).astype(f8)
    woh = (w_fc_proj * SWO).astype(f8)
    wol = (w_fc_proj * SWO - woh.astype(np.float32)).astype(f8)

    id128 = np.eye(128, dtype=f8)
    f8e5 = ml_dtypes.float8_e5m2
    kq = np.arange(128, dtype=np.float32)
    in_maps = []
    for b in range(B):
        xb = x[b]                       # [T, C]
        mu = xb.mean(axis=1)
        var = ((xb - mu[:, None]) ** 2).mean(axis=1)
        r = 1.0 / np.sqrt(var + LN_EPS)
        xh = np.zeros((AUGP * 256, T), dtype=np.float32)
        xh[:C] = (xb * r[:, None]).T
        xh[C] = mu * r
        xh[C + 1] = 1.0
        xh = xh.astype(f8)
        for j in range(4):
            tiles = [j, 7 - j, 8 + j, 15 - j]      # balanced causal q-tiles
            own = np.concatenate(
                [np.arange(t * 128, (t + 1) * 128) for t in tiles])
            xT = np.ascontiguousarray(xb[own].T).astype(bf)  # residual slice
            xho = np.ascontiguousarray(xh[:, own])
            # mskT[q, kt*128+k] = NEG where key (128*kt+k) > query; key tile
            # kt's leading q-block is slot kt//4
            mskT = np.zeros((128, KT * 128), dtype=np.float32)
            for kt in range(KT):
                qglob = 128 * tiles[kt // 4] + kq
                kglob = 128 * kt + kq
                mskT[:, kt * 128:(kt + 1) * 128] = np.where(
                    qglob[:, None] < kglob[None, :], NEG, 0.0)
            in_maps.append({
                "xT": xT, "xh": xh, "xho": xho, "wq": wq, "wk": wk,
                "wv": wv, "wp": wp, "wf": wf, "woh": woh, "wol": wol,
                "mskT": mskT.astype(f8e5), "id128": id128,
            })
    return in_maps


def _get_nc():
    if "nc" not in _cache:
        _cache["nc"] = _build()
    return _cache["nc"]


def _get_runner():
    """Persistent jitted 8-core runner (jit once, call many times)."""
    if "runner" in _cache:
        return _cache["runner"]
    import jax
    import numpy as _np
    from jax.sharding import Mesh, PartitionSpec
    try:
        from jax.experimental.shard_map import shard_map
    except ImportError:
        from jax.shard_map import shard_map
    import concourse.mybir as mybir
    from concourse import bass2jax

    nc = _get_nc()
    bass2jax.install_neuronx_cc_hook()

    partition_name = nc.partition_id_tensor.name if nc.partition_id_tensor else None
    in_names, out_names, out_avals, zero_outs = [], [], [], []
    for alloc in nc.m.functions[0].allocations:
        if not isinstance(alloc, mybir.MemoryLocationSet):
            continue
        name = alloc.memorylocations[0].name
        if alloc.kind == "ExternalInput":
            if name != partition_name:
                in_names.append(name)
        elif alloc.kind == "ExternalOutput":
            shape = tuple(alloc.tensor_shape)
            dtype = mybir.dt.np(alloc.dtype)
            out_names.append(name)
            out_avals.append(jax.core.ShapedArray(shape, dtype))
            zero_outs.append(_np.zeros(shape, dtype))
    n_params = len(in_names)
    n_outs = len(out_avals)
    all_in_names = list(in_names) + list(out_names)
    if partition_name is not None:
        all_in_names.append(partition_name)
    donate = tuple(range(n_params, n_params + n_outs))

    def _body(*args):
        operands = list(args)
        if partition_name is not None:
            operands.append(bass2jax.partition_id_tensor())
        outs = bass2jax._bass_exec_p.bind(
            *operands,
            out_avals=tuple(out_avals),
            in_names=tuple(all_in_names),
            out_names=tuple(out_names),
            lowering_input_output_aliases=(),
            sim_require_finite=True,
            sim_require_nnan=True,
            nc=nc,
        )
        return tuple(outs)

    devices = jax.devices()[:NCORES]
    mesh = Mesh(_np.asarray(devices), ("core",))
    in_specs = (PartitionSpec("core"),) * (n_params + n_outs)
    out_specs = (PartitionSpec("core"),) * n_outs
    sharded = jax.jit(
        shard_map(_body, mesh=mesh, in_specs=in_specs, out_specs=out_specs,
                  check_rep=False),
        donate_argnums=donate, keep_unused=True)

    def run(in_maps):
        concat_in = [
            _np.concatenate([_np.asarray(in_maps[c][n]) for c in range(NCORES)],
                            axis=0)
            for n in in_names
        ]
        concat_zeros = [
            _np.zeros((NCORES * z.shape[0], *z.shape[1:]), z.dtype)
            for z in zero_outs
        ]
        out_arrs = sharded(*concat_in, *concat_zeros)
        return [
            {n: _np.asarray(out_arrs[i]).reshape(NCORES, *out_avals[i].shape)[c]
             for i, n in enumerate(out_names)}
            for c in range(NCORES)
        ]

    _cache["runner"] = run
    return run


def kernel(x, w_attn, w_proj, w_fc, w_fc_proj, ln1_w, ln1_b, ln2_w, ln2_b):
    x = np.asarray(x, dtype=np.float32)
    in_maps = _prep_inputs(
        x, np.asarray(w_attn, np.float32), np.asarray(w_proj, np.float32),
        np.asarray(w_fc, np.float32), np.asarray(w_fc_proj, np.float32),
        np.asarray(ln1_w, np.float32), np.asarray(ln1_b, np.float32),
        np.asarray(ln2_w, np.float32), np.asarray(ln2_b, np.float32))
    results = _get_runner()(in_maps)
    out = np.empty((B, T, C), dtype=np.float32)
    for c in range(NCORES):
        b, j = c // 4, c % 4
        res = results[c]["outT"].T                 # [Q, C], slot-ordered
        for s, t in enumerate([j, 7 - j, 8 + j, 15 - j]):
            out[b, t * 128:(t + 1) * 128, :] = res[s * 128:(s + 1) * 128]
    return out


# revision 44
# speedup vs baseline: 1.0494x; 1.0494x over previous
"""Trainium2 Bass kernel for a dense transformer block (pre-LN GPT block).

Reference computation (fp32, B=2, T=2048, C=1024, H=16 heads, FFN 4C):
    x = x + attn(LN1(x)) ; x = x + mlp(LN2(x))   (causal attention, tanh-gelu)

Distribution (8 NeuronCores, no collectives):
  - batch split (2) x sequence split (4): core c handles batch b=c//4,
    query quarter j=c%4 (512 tokens).
  - K/V projections are computed for the full 2048-token batch on every
    core of the group (replicated); everything else (Q, attention rows,
    proj, LN2, FFN, residuals) is token-local.
  - causality via host-built masks: tokens are rotated per-core so the
    own 512 tokens come first in the key order; the 512x512 diagonal
    block uses on-the-fly iota masks; the remaining key tiles are
    uniformly allowed/denied per core by zeroing V rows (and their
    softmax-denominator ones-column) via the m01 input.

Precision strategy: all projection matmuls (QKV, attn-proj, FFN fc and
proj) run in fp8e4 with DoubleRow perf mode (two 128-row contraction
tiles per instruction); the AV matmul likewise (A=exp(s-3) and V are
quantized to fp8e4; the -3 bias keeps A under fp8e4's 240 max and
cancels in the softmax ratio because the denominator rides as a 65th
ones-column of V through the same fp8 pipeline). Score matmuls (QK^T)
stay bf16 for softmax accuracy. Weights are pre-scaled by powers of two
on the host so fp8 sees unit-variance values; the inverse scales fold
into existing per-tile scalar passes. LN is folded into the following
matmul via augmented contraction rows; contraction is padded 9->10
tiles (5 DoubleRow pairs).
"""

import math
import numpy as np
import ml_dtypes

B, T, C = 2, 2048, 1024
H, DH = 16, 64
F = 4 * C
Q = 512          # query tokens per core
NCORES = 8
KT = T // 128    # 16 key tiles
CT = C // 128    # 8 feature tiles
CP = CT // 2     # 4 feature pair-tiles
AUGP = 5         # contraction pair-tiles incl. LN-fold aug pair (10 x 128)
FT = F // 128    # 32 ffn tiles
FP = FT // 2     # 16 ffn pair-tiles
LN_EPS = 1e-5
NEG = -30000.0
EXP_BIAS = -3.0  # exp(s-3): keeps A < fp8e4 max; cancels in softmax ratio
SW = 32.0        # host pre-scale on wk/wv/wp/wf (power of 2)
SWQ = 256.0      # host pre-scale on wq (carries 1/sqrt(DH))
SWO = 64.0       # host pre-scale on wo

_cache = {}


def _build():
    import concourse.mybir as mybir
    import concourse.tile as tile
    from concourse import bacc

    f32 = mybir.dt.float32
    bf16 = mybir.dt.bfloat16
    f8 = mybir.dt.float8e4
    Alu = mybir.AluOpType
    Act = mybir.ActivationFunctionType
    DR = mybir.MatmulPerfMode.DoubleRow

    nc = bacc.Bacc("TRN2", target_bir_lowering=False, debug=False,
                   num_devices=NCORES)

    # pair-tile operands are packed host-side as [n, 128, 2, X] so each
    # [128, 2, X] tile loads with ONE dma of 2X-byte descriptors
    xT_d = nc.dram_tensor("xT", [C, Q], bf16, kind="ExternalInput")
    xh_d = nc.dram_tensor("xh", [AUGP, 128, 2, T], f8, kind="ExternalInput")
    xho_d = nc.dram_tensor("xho", [AUGP, 128, 2, Q], f8, kind="ExternalInput")
    wq_d = nc.dram_tensor("wq", [AUGP, 128, 2, C], f8, kind="ExternalInput")
    wk_d = nc.dram_tensor("wk", [AUGP, 128, 2, C], f8, kind="ExternalInput")
    wv_d = nc.dram_tensor("wv", [AUGP, 128, 2, C], f8, kind="ExternalInput")
    wp_d = nc.dram_tensor("wp", [CP, 128, 2, C], f8, kind="ExternalInput")
    wf_d = nc.dram_tensor("wf", [(2 * CP + 1) * 128, F], bf16,
                          kind="ExternalInput")
    woh_d = nc.dram_tensor("woh", [FP, 128, 2, C], f8, kind="ExternalInput")
    wol_d = nc.dram_tensor("wol", [FP, 128, 2, C], f8, kind="ExternalInput")
    mskT_d = nc.dram_tensor("mskT", [128, KT * 128], mybir.dt.float8e5,
                            kind="ExternalInput")
    id_d = nc.dram_tensor("id128", [128, 128], f8, kind="ExternalInput")
    out_d = nc.dram_tensor("outT", [C, Q], f32, kind="ExternalOutput")

    # suffix trim: key tile kt feeds query columns [QS[kt], Q)
    QS = [128 * (kt // 4) for kt in range(KT)]

    with tile.TileContext(nc) as tc:
        cst = tc.alloc_tile_pool(name="cst", bufs=1, side="left")
        ones_col = cst.tile([128, 1], bf16, name="ones_col", tag="ones_col")
        ones_r128 = cst.tile([1, 128], f32, name="ones_r128", tag="ones_r128")
        ones_r64b = cst.tile([1, 64], bf16, name="ones_r64b", tag="ones_r64b")
        eps_t = cst.tile([1, 1], f32, name="eps", tag="eps")
        expb_t = cst.tile([128, 1], f32, name="expb", tag="expb")
        nc.vector.memset(ones_col[:], 1.0)
        nc.vector.memset(ones_r128[:], 1.0)
        nc.vector.memset(ones_r64b[:], 1.0)
        nc.vector.memset(eps_t[:], LN_EPS)
        nc.vector.memset(expb_t[:], EXP_BIAS)

        p_ytil = tc.alloc_tile_pool(name="ytil", bufs=1, side="left")
        ytil = [p_ytil.tile([128, 2, Q], f8, name=f"ytil{p}", tag=f"ytil{p}")
                for p in range(CP)]

        kqv = tc.alloc_tile_pool(name="kqv", bufs=1, side="left")
        kT_sb = [kqv.tile([128, T], bf16, name=f"kT{m}", tag=f"kT{m}")
                 for m in range(CT)]
        qT_sb = [kqv.tile([128, Q], bf16, name=f"qT{m}", tag=f"qT{m}")
                 for m in range(CT)]
        v_sb = [kqv.tile([128, 2, H, DH + 1], f8, name=f"v{t}", tag=f"v{t}")
                for t in range(KT // 2)]
        mskT_sb = kqv.tile([128, KT * 128], mybir.dt.float8e5, name="mskT",
                           tag="mskT")
        id_sb = kqv.tile([128, 128], f8, name="id128", tag="id128")
        # softmax-denominator ones column (masked keys zero out via exp->0)
        for tp in range(KT // 2):
            nc.vector.memset(v_sb[tp][:, :, :, DH:DH + 1], 1.0)

        p_xhat = tc.alloc_tile_pool(name="xhat", bufs=1, side="left")
        xhat = [p_xhat.tile([128, 2, T], f8, name=f"xh{k}", tag=f"xh{k}")
                for k in range(AUGP)]
        xho_sb = [p_xhat.tile([128, 2, Q], f8, name=f"xho{k}", tag=f"xho{k}")
                  for k in range(AUGP)]

        # QKV weights (left, release order: wv -> wq -> wk)
        p_wk = tc.alloc_tile_pool(name="wkp", bufs=1, side="left")
        wk_sb = [p_wk.tile([128, 2, C], f8, name=f"wk{k}", tag=f"wk{k}")
                 for k in range(AUGP)]
        p_wq = tc.alloc_tile_pool(name="wqp", bufs=1, side="left")
        wq_sb = [p_wq.tile([128, 2, C], f8, name=f"wq{k}", tag=f"wq{k}")
                 for k in range(AUGP)]
        p_wv = tc.alloc_tile_pool(name="wvp", bufs=1, side="left")
        wv_sb = [p_wv.tile([128, 2, C], f8, name=f"wv{k}", tag=f"wv{k}")
                 for k in range(AUGP)]

        # proj weights (right): loaded up front, consumed in phase 3
        p_wp = tc.alloc_tile_pool(name="wpp", bufs=1, side="right")
        wp_sb = [p_wp.tile([128, 2, C], f8, name=f"wp{k}", tag=f"wp{k}")
                 for k in range(CP)]

        # input DMA in priority order for the first attention pair:
        # xh -> wk -> Q-path -> masks -> wv -> wp; one dma per pair tile
        for k in range(AUGP):
            nc.sync.dma_start(xhat[k][:], xh_d[k])
        for k in range(AUGP):
            nc.sync.dma_start(wk_sb[k][:], wk_d[k])
        for k in range(AUGP):
            nc.sync.dma_start(xho_sb[k][:], xho_d[k])
            nc.sync.dma_start(wq_sb[k][:], wq_d[k])
        nc.sync.dma_start(mskT_sb[:], mskT_d[:])
        nc.sync.dma_start(id_sb[:], id_d[:])
        for k in range(AUGP):
            nc.sync.dma_start(wv_sb[k][:], wv_d[k])
        for k in range(CP):
            nc.sync.dma_start(wp_sb[k][:], wp_d[k])

        # ffn weights, part 1: allocated up front so the DMA can run during
        # the attention tail (6 of 9 bf16 tiles; the rest load in phase 3)
        AUG9 = 2 * CP + 1
        p_wf1 = tc.alloc_tile_pool(name="wfp1", bufs=1, side="right")
        wf1 = [p_wf1.tile([128, F], bf16, name=f"wf{k}", tag=f"wf{k}")
               for k in range(6)]

        # ---- attention: K/Q/V projections pipelined into the head loop ----
        with tc.tile_pool(name="pa", bufs=4, side="right") as p_a, \
             tc.tile_pool(name="prl", bufs=1, side="right") as p_rl, \
             tc.tile_pool(name="pqkv", bufs=2, space="PSUM") as pq, \
             tc.tile_pool(name="ps2", bufs=2, space="PSUM") as ps2, \
             tc.tile_pool(name="py", bufs=2, space="PSUM") as py:

            def v_chunk(n, t):
                # V proj of key tile t for head-half n (v dims [n*512,+512))
                ns = slice(n * 512, (n + 1) * 512)
                ts_ = slice(t * 128, (t + 1) * 128)
                ps = pq.tile([128, 8, 64], f32, name="pk", tag="pk")
                for k in range(AUGP):
                    nc.tensor.matmul(ps[:], xhat[k][:, :, ts_],
                                     wv_sb[k][:, :, ns],
                                     start=(k == 0), stop=(k == AUGP - 1),
                                     perf_mode=DR)
                nc.vector.tensor_scalar_mul(
                    v_sb[t // 2][:, t % 2, n * 8:(n + 1) * 8, 0:DH], ps[:],
                    1.0 / SW)

            def k_chunk(m, n):
                ns = slice(n * 512, (n + 1) * 512)
                ps = pq.tile([128, 512], f32, name="pk", tag="pk")
                for k in range(AUGP):
                    nc.tensor.matmul(ps[:], wk_sb[k][:, :, m * 128:(m + 1) * 128],
                                     xhat[k][:, :, ns],
                                     start=(k == 0), stop=(k == AUGP - 1),
                                     perf_mode=DR)
                nc.vector.tensor_scalar_mul(kT_sb[m][:, ns], ps[:], 1.0 / SW)

            def q_chunk(m):
                ps = pq.tile([128, 512], f32, name="pk", tag="pk")
                for k in range(AUGP):
                    nc.tensor.matmul(ps[:], wq_sb[k][:, :, m * 128:(m + 1) * 128],
                                     xho_sb[k][:],
                                     start=(k == 0), stop=(k == AUGP - 1),
                                     perf_mode=DR)
                nc.vector.tensor_scalar_mul(qT_sb[m][:], ps[:], 1.0 / SWQ)

            pending = []

            def pump():
                if pending:
                    pending.pop(0)()

            def attention_head(h):
                kt_tile = h // 2
                po = (h % 2) * 64
                yb = py.tile([128, 512], f32, name="y", tag="y")
                for tp in range(KT // 2):        # key-tile pairs
                    pump()
                    pump()
                    qs = QS[2 * tp]
                    s_ps = ps2.tile([128, 2, 512], f32, name="s", tag="s")
                    a_sb = p_a.tile([128, 2, 512], f8, name="a", tag="a")
                    for half in range(2):
                        t = tp * 2 + half
                        # leading 128-col block: host-built causal/padding
                        # mask lands in psum via a tiny matmul, then scores
                        # accumulate on top; the suffix is mask-free.
                        nc.tensor.matmul(
                            s_ps[:, half, qs:qs + 128],
                            mskT_sb[:, t * 128:(t + 1) * 128], id_sb[:],
                            start=True, stop=False, skip_group_check=True)
                        nc.tensor.matmul(
                            s_ps[:, half, qs:qs + 128],
                            kT_sb[kt_tile][po:po + 64, t * 128:(t + 1) * 128],
                            qT_sb[kt_tile][po:po + 64, qs:qs + 128],
                            start=False, stop=True, skip_group_check=True)
                        if qs + 128 < Q:
                            nc.tensor.matmul(
                                s_ps[:, half, qs + 128:],
                                kT_sb[kt_tile][po:po + 64,
                                               t * 128:(t + 1) * 128],
                                qT_sb[kt_tile][po:po + 64, qs + 128:],
                                start=True, stop=True, skip_group_check=True)
                    nc.scalar.activation(a_sb[:, :, qs:], s_ps[:, :, qs:],
                                         Act.Exp, bias=expb_t[:])
                    nc.tensor.matmul(yb[0:65, qs:], v_sb[tp][:, :, h, :],
                                     a_sb[:, :, qs:],
                                     start=(tp == 0), stop=(tp == KT // 2 - 1),
                                     perf_mode=DR, skip_group_check=True)
                rl = p_rl.tile([1, 512], bf16, name="rl", tag="rl")
                rlf = p_rl.tile([1, 512], f32, name="rlf", tag="rlf")
                nc.vector.reciprocal(rlf[:], yb[64:65, :])
                nc.vector.tensor_copy(rl[:], rlf[:])
                nc.tensor.matmul(yb[64:128, :], ones_r64b[:], rl[:],
                                 start=True, stop=True)
                rlb = p_rl.tile([64, 512], bf16, name="rlb", tag="rlb")
                nc.vector.tensor_copy(rlb[:], yb[64:128, :])
                # head h -> ytil pair p=h//4, slot (h//2)%2, rows 64*(h%2)
                nc.vector.tensor_tensor(
                    ytil[h // 4][64 * (h % 2):64 * (h % 2) + 64,
                                 (h // 2) % 2, :],
                    yb[0:64, :], rlb[:], Alu.mult)

            # prologue: K/Q for m=0 (gating head 0), then first V0 tiles
            for n in range(4):
                k_chunk(0, n)
            q_chunk(0)
            for t in range(4):
                v_chunk(0, t)
            pending += [lambda t=t: v_chunk(0, t) for t in range(4, KT)]
            V1_SCHED = {1: range(0, 6), 2: range(6, 12), 3: range(12, 16)}
            for m in range(CT):
                if m < CT - 1:
                    pending.extend(
                        [lambda n=n, m1=m + 1: k_chunk(m1, n) for n in range(4)]
                        + [lambda m1=m + 1: q_chunk(m1)])
                for t in V1_SCHED.get(m, ()):
                    pending.append(lambda t=t: v_chunk(1, t))
                attention_head(2 * m)
                attention_head(2 * m + 1)
                while pending:
                    pending.pop(0)()
                if m == 4:
                    # prefetch most of wf (bf16, 6 of 9 tiles) while the
                    # tail attention iterations run
                    for k in range(6):
                        nc.sync.dma_start(wf1[k][:],
                                          wf_d[k * 128:(k + 1) * 128, :])
        p_wv.release()
        p_wq.release()
        p_wk.release()
        p_xhat.release()
        kqv.release()

        # ffn weights, part 2 (tiles 6-8): loaded during phase 3
        p_wf2 = tc.alloc_tile_pool(name="wfp2", bufs=1, side="right")
        wf_sb = wf1 + [p_wf2.tile([128, F], bf16, name=f"wf{k}", tag=f"wf{k}")
                       for k in range(6, AUG9)]

        # ------------ phase 3: proj + residual + LN2 ------------
        with tc.tile_pool(name="p34", bufs=1, side="right") as p34, \
             tc.tile_pool(name="p3s", bufs=2, side="right") as p3s:
            x2_sb = [p34.tile([128, Q], f32, name=f"x2{m}", tag=f"x2{m}")
                     for m in range(CT)]
            x2b = [p34.tile([128, Q], bf16, name=f"x2b{m}", tag=f"x2b{m}")
                   for m in range(CT)]
            xh2a = p34.tile([128, Q], bf16, name="xh2a", tag="xh2a")
            mu2 = p34.tile([1, Q], f32, name="mu2", tag="mu2")
            e22 = p34.tile([1, Q], f32, name="e22", tag="e22")
            rr2 = p34.tile([1, Q], f32, name="rr2", tag="rr2")
            mur2 = p34.tile([1, Q], f32, name="mur2", tag="mur2")
            r2b = p34.tile([128, Q], f32, name="r2b", tag="r2b")

            with tc.tile_pool(name="pxq", bufs=1, side="right") as p_xq:
                xq_sb = [p_xq.tile([128, Q], bf16, name=f"xq{m}", tag=f"xq{m}")
                         for m in range(CT)]
                for m in range(CT):
                    nc.sync.dma_start(xq_sb[m][:],
                                      xT_d[m * 128:(m + 1) * 128, :])
                for k in range(6, AUG9):
                    nc.sync.dma_start(wf_sb[k][:],
                                      wf_d[k * 128:(k + 1) * 128, :])
                with tc.tile_pool(name="pp3", bufs=4, space="PSUM") as pp3, \
                     tc.tile_pool(name="pst2", bufs=1, space="PSUM") as pst2:
                    s2_ps = pst2.tile([1, Q], f32, name="s2", tag="s2")
                    q2_ps = pst2.tile([1, Q], f32, name="q2", tag="q2")
                    for m in range(CT):
                        ms = slice(m * 128, (m + 1) * 128)
                        ps = pp3.tile([128, Q], f32, name="pj", tag="pj")
                        for k in range(CP):
                            nc.tensor.matmul(ps[:], wp_sb[k][:, :, ms],
                                             ytil[k][:],
                                             start=(k == 0), stop=(k == CP - 1),
                                             perf_mode=DR)
                        nc.vector.scalar_tensor_tensor(
                            x2_sb[m][:], ps[:], 1.0 / SW, xq_sb[m][:],
                            Alu.mult, Alu.add)
                        nc.vector.tensor_copy(x2b[m][:], x2_sb[m][:])
                        sqt = p3s.tile([128, Q], bf16, name="sq", tag="sq")
                        nc.scalar.square(sqt[:], x2b[m][:])
                        nc.tensor.matmul(s2_ps[:], ones_col[:], x2b[m][:],
                                         start=(m == 0), stop=(m == CT - 1))
                        nc.tensor.matmul(q2_ps[:], ones_col[:], sqt[:],
                                         start=(m == 0), stop=(m == CT - 1))
                    nc.vector.tensor_scalar_mul(mu2[:], s2_ps[:], 1.0 / C)
                    nc.vector.tensor_scalar_mul(e22[:], q2_ps[:], 1.0 / C)
            nc.vector.tensor_tensor(rr2[:], mu2[:], mu2[:], Alu.mult)
            nc.vector.tensor_tensor(rr2[:], e22[:], rr2[:], Alu.subtract)
            nc.scalar.activation(rr2[:], rr2[:], Act.Sqrt, bias=eps_t[:])
            nc.vector.reciprocal(rr2[:], rr2[:])
            nc.vector.tensor_tensor(mur2[:], mu2[:], rr2[:], Alu.mult)
            with tc.tile_pool(name="pbc2", bufs=1, space="PSUM") as pbc2:
                b_ps = pbc2.tile([128, Q], f32, name="b2", tag="b2")
                nc.tensor.matmul(b_ps[:], ones_r128[:], rr2[:],
                                 start=True, stop=True)
                nc.scalar.copy(r2b[:], b_ps[:])
            for k in range(CT):
                nc.vector.tensor_tensor(x2b[k][:], x2b[k][:], r2b[:], Alu.mult)
            nc.vector.memset(xh2a[:], 0.0)
            nc.vector.memset(xh2a[0:2, :], 1.0)
            nc.vector.tensor_copy(xh2a[0:1, :], mur2[:])
            xhat2 = x2b + [xh2a]

            # ------------ phase 4: FFN ------------
            # fc in bf16; gelu output split hg = hgh + hgl (both fp8e4) so
            # the fc-proj runs as three fp8 DoubleRow chains:
            #   o = hgh@(woh+wol) + hgl@woh   (wol = residual of woh quant)
            with tc.tile_pool(name="p4", bufs=1, side="right") as p4, \
                 tc.tile_pool(name="p4b", bufs=4, side="right") as p4b:
                hgh_sb = [p4.tile([128, 2, Q], f8, name=f"hgh{p}",
                                  tag=f"hgh{p}") for p in range(FP)]
                hgl_sb = [p4.tile([128, 2, Q], f8, name=f"hgl{p}",
                                  tag=f"hgl{p}") for p in range(FP)]
                with tc.tile_pool(name="ph", bufs=6, space="PSUM") as ph:
                    for m in range(FT):
                        ms = slice(m * 128, (m + 1) * 128)
                        ps = ph.tile([128, Q], f32, name="h", tag="h")
                        for k in range(AUG9):
                            nc.tensor.matmul(ps[:], wf_sb[k][:, ms],
                                             xhat2[k][:],
                                             start=(k == 0),
                                             stop=(k == AUG9 - 1))
                        hgb = p4b.tile([128, Q], bf16, name="hgb", tag="hgb")
                        nc.scalar.activation(hgb[:], ps[:],
                                             Act.Gelu_apprx_tanh)
                        hi = hgh_sb[m // 2][:, m % 2, :]
                        nc.vector.tensor_copy(hi, hgb[:])
                        nc.vector.scalar_tensor_tensor(
                            hgl_sb[m // 2][:, m % 2, :], hi, -1.0, hgb[:],
                            Alu.mult, Alu.add)
                with tc.tile_pool(name="pwo", bufs=4, side="right") as p_wo, \
                     tc.tile_pool(name="pwol", bufs=1, side="right") as p_wol, \
                     tc.tile_pool(name="pout", bufs=2, side="right") as p_out, \
                     tc.tile_pool(name="po", bufs=1, space="PSUM") as po:
                    o_ps = [po.tile([128, Q], f32, name=f"o{m}", tag=f"o{m}")
                            for m in range(CT)]
                    wol_t = [p_wol.tile([128, 2, C], f8, name=f"wol{kp}",
                                        tag=f"wol{kp}") for kp in range(FP)]
                    for kp in range(FP):
                        wo_t = p_wo.tile([128, 2, C], f8, name="wo", tag="wo")
                        nc.sync.dma_start(wo_t[:], woh_d[kp])
                        nc.sync.dma_start(wol_t[kp][:], wol_d[kp])
                        for m in range(CT):
                            nc.tensor.matmul(o_ps[m][:],
                                             wo_t[:, :, m * 128:(m + 1) * 128],
                                             hgh_sb[kp][:],
                                             start=(kp == 0), stop=False,
                                             perf_mode=DR,
                                             skip_group_check=True)
                        for m in range(CT):
                            nc.tensor.matmul(o_ps[m][:],
                                             wo_t[:, :, m * 128:(m + 1) * 128],
                                             hgl_sb[kp][:],
                                             start=False, stop=False,
                                             perf_mode=DR,
                                             skip_group_check=True)
                    # final chain m-outer so each output column block drains
                    # (residual add + store) while later blocks still matmul
                    for m in range(CT):
                        for kp in range(FP):
                            nc.tensor.matmul(o_ps[m][:],
                                             wol_t[kp][:, :,
                                                       m * 128:(m + 1) * 128],
                                             hgh_sb[kp][:],
                                             start=False, stop=(kp == FP - 1),
                                             perf_mode=DR,
                                             skip_group_check=True)
                        ot = p_out.tile([128, Q], f32, name="ot", tag="ot")
                        nc.vector.scalar_tensor_tensor(
                            ot[:], o_ps[m][:], 1.0 / SWO, x2_sb[m][:],
                            Alu.mult, Alu.add)
                        nc.sync.dma_start(out_d[m * 128:(m + 1) * 128, :], ot[:])

        p_wf2.release()
        p_wf1.release()
        p_wp.release()
        p_ytil.release()
        cst.release()

    nc.compile()
    return nc


def _prep_inputs(x, w_attn, w_proj, w_fc, w_fc_proj, ln1_w, ln1_b, ln2_w, ln2_b):
    f8 = ml_dtypes.float8_e4m3
    bf = ml_dtypes.bfloat16
    scale = 1.0 / math.sqrt(DH)

    def pack2(a):
        # [n*256, X] -> [n, 128, 2, X]: one contiguous 2X-byte dma
        # descriptor per partition row
        n = a.shape[0] // 256
        return np.ascontiguousarray(
            a.reshape(n, 2, 128, a.shape[1]).swapaxes(1, 2))

    def aug(W, lw, lb, s, rows, dt):
        out = np.zeros((rows, W.shape[1]), dtype=np.float32)
        Ws = lw[:, None] * W * s
        out[:C] = Ws
        out[C] = -Ws.sum(axis=0)
        out[C + 1] = (lb * s) @ W
        return out.astype(dt)

    wq = pack2(aug(w_attn[:, :C] * scale, ln1_w, ln1_b, SWQ, AUGP * 256, f8))
    wk = pack2(aug(w_attn[:, C:2 * C], ln1_w, ln1_b, SW, AUGP * 256, f8))
    wv = pack2(aug(w_attn[:, 2 * C:], ln1_w, ln1_b, SW, AUGP * 256, f8))
    wf = aug(w_fc, ln2_w, ln2_b, 1.0, (2 * CP + 1) * 128, bf)
    wp = pack2((w_proj * SW).astype(f8))
    woh = (w_fc_proj * SWO).astype(f8)
    wol = pack2((w_fc_proj * SWO - woh.astype(np.float32)).astype(f8))
    woh = pack2(woh)

    id128 = np.eye(128, dtype=f8)
    f8e5 = ml_dtypes.float8_e5m2
    kq = np.arange(128, dtype=np.float32)
    in_maps = []
    for b in range(B):
        xb = x[b]                       # [T, C]
        mu = xb.mean(axis=1)
        var = ((xb - mu[:, None]) ** 2).mean(axis=1)
        r = 1.0 / np.sqrt(var + LN_EPS)
        xh = np.zeros((AUGP * 256, T), dtype=np.float32)
        xh[:C] = (xb * r[:, None]).T
        xh[C] = mu * r
        xh[C + 1] = 1.0
        xh = xh.astype(f8)
        xh_p = pack2(xh)
        for j in range(4):
            tiles = [j, 7 - j, 8 + j, 15 - j]      # balanced causal q-tiles
            own = np.concatenate(
                [np.arange(t * 128, (t + 1) * 128) for t in tiles])
            xT = np.ascontiguousarray(xb[own].T).astype(bf)  # residual slice
            xho = pack2(np.ascontiguousarray(xh[:, own]))
            # mskT[q, kt*128+k] = NEG where key (128*kt+k) > query; key tile
            # kt's leading q-block is slot kt//4
            mskT = np.zeros((128, KT * 128), dtype=np.float32)
            for kt in range(KT):
                qglob = 128 * tiles[kt // 4] + kq
                kglob = 128 * kt + kq
                mskT[:, kt * 128:(kt + 1) * 128] = np.where(
                    qglob[:, None] < kglob[None, :], NEG, 0.0)
            in_maps.append({
                "xT": xT, "xh": xh_p, "xho": xho, "wq": wq, "wk": wk,
                "wv": wv, "wp": wp, "wf": wf, "woh": woh, "wol": wol,
                "mskT": mskT.astype(f8e5), "id128": id128,
            })
    return in_maps


def _get_nc():
    if "nc" not in _cache:
        _cache["nc"] = _build()
    return _cache["nc"]


def _get_runner():
    """Persistent jitted 8-core runner (jit once, call many times)."""
    if "runner" in _cache:
        return _cache["runner"]
    import jax
    import numpy as _np
    from jax.sharding import Mesh, PartitionSpec
    try:
        from jax.experimental.shard_map import shard_map
    except ImportError:
        from jax.shard_map import shard_map
    import concourse.mybir as mybir
    from concourse import bass2jax

    nc = _get_nc()
    bass2jax.install_neuronx_cc_hook()

    partition_name = nc.partition_id_tensor.name if nc.partition_id_tensor else None
    in_names, out_names, out_avals, zero_outs = [], [], [], []
    for alloc in nc.m.functions[0].allocations:
        if not isinstance(alloc, mybir.MemoryLocationSet):
            continue
        name = alloc.memorylocations[0].name
        if alloc.kind == "ExternalInput":
            if name != partition_name:
                in_names.append(name)
        elif alloc.kind == "ExternalOutput":
            shape = tuple(alloc.tensor_shape)
            dtype = mybir.dt.np(alloc.dtype)
            out_names.append(name)
            out_avals.append(jax.core.ShapedArray(shape, dtype))
            zero_outs.append(_np.zeros(shape, dtype))
    n_params = len(in_names)
    n_outs = len(out_avals)
    all_in_names = list(in_names) + list(out_names)
    if partition_name is not None:
        all_in_names.append(partition_name)
    donate = tuple(range(n_params, n_params + n_outs))

    def _body(*args):
        operands = list(args)
        if partition_name is not None:
            operands.append(bass2jax.partition_id_tensor())
        outs = bass2jax._bass_exec_p.bind(
            *operands,
            out_avals=tuple(out_avals),
            in_names=tuple(all_in_names),
            out_names=tuple(out_names),
            lowering_input_output_aliases=(),
            sim_require_finite=True,
            sim_require_nnan=True,
            nc=nc,
        )
        return tuple(outs)

    devices = jax.devices()[:NCORES]
    mesh = Mesh(_np.asarray(devices), ("core",))
    in_specs = (PartitionSpec("core"),) * (n_params + n_outs)
    out_specs = (PartitionSpec("core"),) * n_outs
    sharded = jax.jit(
        shard_map(_body, mesh=mesh, in_specs=in_specs, out_specs=out_specs,
                  check_rep=False),
        donate_argnums=donate, keep_unused=True)

    def run(in_maps):
        concat_in = [
            _np.concatenate([_np.asarray(in_maps[c][n]) for c in range(NCORES)],
                            axis=0)
            for n in in_names
        ]
        concat_zeros = [
            _np.zeros((NCORES * z.shape[0], *z.shape[1:]), z.dtype)
            for z in zero_outs
        ]
        out_arrs = sharded(*concat_in, *concat_zeros)
        return [
            {n: _np.asarray(out_arrs[i]).reshape(NCORES, *out_avals[i].shape)[c]
             for i, n in enumerate(out_names)}
            for c in range(NCORES)
        ]

    _cache["runner"] = run
    return run


def kernel(x, w_attn, w_proj, w_fc, w_fc_proj, ln1_w, ln1_b, ln2_w, ln2_b):
    x = np.asarray(x, dtype=np.float32)
    in_maps = _prep_inputs(
        x, np.asarray(w_attn, np.float32), np.asarray(w_proj, np.float32),
        np.asarray(w_fc, np.float32), np.asarray(w_fc_proj, np.float32),
        np.asarray(ln1_w, np.float32), np.asarray(ln1_b, np.float32),
        np.asarray(ln2_w, np.float32), np.asarray(ln2_b, np.float32))
    results = _get_runner()(in_maps)
    out = np.empty((B, T, C), dtype=np.float32)
    for c in range(NCORES):
        b, j = c // 4, c % 4
        res = results[c]["outT"].T                 # [Q, C], slot-ordered
        for s, t in enumerate([j, 7 - j, 8 + j, 15 - j]):
            out[b, t * 128:(t + 1) * 128, :] = res[s * 128:(s + 1) * 128]
    return out
